# revision 2
# baseline (speedup 1.0000x reference)
"""Bass/Tile TRN2 kernel for additive-attention pooling.

Math per sample s:
    e = tanh(x[s] @ W + b)          # (T, 1)
    a = softmax(e, axis=0)          # over T
    y[s] = sum_t a[t] * x[s, t, :]  # (U,)

tanh is bounded in (-1, 1), so softmax needs no max-subtraction:
    p = exp(e);  y[s] = (sum_t p[t] x[s,t]) / (sum_t p[t])

Sharding: data-parallel over batch across 8 NeuronCores (32 samples each).

Per-core dataflow (v2). x is streamed once, one whole sample (4 MiB) per
DMA, via the SWDGE (gpsimd) path which casts fp32 -> bf16 in the DMA
datapath. Layout is q-packed (t = p*16 + q), so partition p receives 16
contiguous HBM rows (32 KiB) per sample. Per sample:
  - 16x DVE scalar_tensor_tensor over bf16 [128,512] slices (2x_1P mode)
    with accum_out -> e column per q
  - DVE adds bias b; ACT tanh; ACT exp (out bf16) with accum_out row sums
  - 16 accumulating bf16 matmuls (K=128, M=1, N=512) -> wsum [1,512] PSUM
  - denominator via ones-matmul of the ACT row sums, DVE reciprocal,
    ACT scaled copy, DMA out
bf16 everywhere halves DVE time (the co-bottleneck at fp32) and makes the
matmuls 1 cycle/row; rel-err budget (2e-2) dwarfs bf16 rounding (~3e-3).
"""

from contextlib import ExitStack

import numpy as np

B, T, U = 256, 2048, 512
N_CORES = 8
B_LOC = B // N_CORES
P = 128

_BUILD_CACHE = {}


def _emit(ctx, tc, x, W, b, y, xbufs):
    from concourse import mybir

    nc = tc.nc
    f32 = mybir.dt.float32
    bf16 = mybir.dt.bfloat16
    Alu = mybir.AluOpType
    Act = mybir.ActivationFunctionType

    b_loc, t_len, u = x.shape
    Q = t_len // P            # timesteps per partition (16)

    const = ctx.enter_context(tc.tile_pool(name="const", bufs=1))
    xp = ctx.enter_context(tc.tile_pool(name="xp", bufs=xbufs))
    scr_p = ctx.enter_context(tc.tile_pool(name="scr", bufs=2))
    ep = ctx.enter_context(tc.tile_pool(name="ep", bufs=4))
    sp = ctx.enter_context(tc.tile_pool(name="sp", bufs=6))
    op = ctx.enter_context(tc.tile_pool(name="op", bufs=4))
    ps_wb = ctx.enter_context(tc.tile_pool(name="ps_wb", bufs=1, space="PSUM"))
    ps_w = ctx.enter_context(tc.tile_pool(name="ps_w", bufs=4, space="PSUM"))
    ps_s = ctx.enter_context(tc.tile_pool(name="ps_s", bufs=2, space="PSUM"))

    # ---- constants ----
    # W as a [1, U] row, broadcast to all 128 partitions via a K=1 matmul.
    w_row = const.tile([1, u], f32)
    nc.sync.dma_start(w_row[:], W.rearrange("u o -> o u"))
    ones_row = const.tile([1, P], f32)
    nc.vector.memset(ones_row[:], 1.0)
    ones_col = const.tile([P, 1], f32)
    nc.vector.memset(ones_col[:], 1.0)
    wb_ps = ps_wb.tile([P, u], f32, tag="wb_ps")
    nc.tensor.matmul(wb_ps[:], ones_row[:], w_row[:], start=True, stop=True)
    Wb = const.tile([P, u], bf16)
    nc.vector.tensor_copy(Wb[:], wb_ps[:])
    # b rearranged to [partition, q] matching the q-packed x layout
    bt = const.tile([P, Q], f32)
    nc.sync.dma_start(bt[:], b.rearrange("(p q) o -> p (q o)", p=P, q=Q))

    xr = x.rearrange("s (p q) u -> s p (q u)", p=P, q=Q)

    for s in range(b_loc):
        # whole sample: partition p holds 16 contiguous HBM rows, cast to
        # bf16 in the SWDGE DMA datapath (4 MiB read -> 2 MiB SBUF write)
        xt = xp.tile([P, Q * u], bf16)
        nc.gpsimd.dma_start(xt[:], xr[s])
        e = ep.tile([P, Q], f32, tag="e")
        for q in range(Q):
            scr = scr_p.tile([P, u], bf16)
            nc.vector.scalar_tensor_tensor(
                out=scr[:],
                in0=xt[:, q * u:(q + 1) * u],
                scalar=1.0,
                in1=Wb[:],
                op0=Alu.mult,
                op1=Alu.mult,
                accum_out=e[:, q:q + 1],
            )
        eb = ep.tile([P, Q], f32, tag="eb")
        nc.vector.tensor_add(eb[:], e[:], bt[:])
        th = ep.tile([P, Q], f32, tag="th")
        nc.scalar.activation(th[:], eb[:], Act.Tanh)
        p_sc = ep.tile([P, Q], bf16, tag="p_sc")
        rs = sp.tile([P, 1], f32, tag="rs")
        nc.scalar.activation(p_sc[:], th[:], Act.Exp, accum_out=rs[:])

        # weighted sum: 16 accumulating bf16 matmuls into one PSUM row
        wsum = ps_w.tile([1, u], f32, tag="wsum")
        for q in range(Q):
            nc.tensor.matmul(
                wsum[:],
                p_sc[:, q:q + 1],
                xt[:, q * u:(q + 1) * u],
                start=(q == 0), stop=(q == Q - 1),
            )

        # denominator: s = rs.T @ ones
        s_ps = ps_s.tile([1, 1], f32)
        nc.tensor.matmul(s_ps[:], rs[:], ones_col[:], start=True, stop=True)
        inv = sp.tile([1, 1], f32, tag="inv")
        nc.vector.reciprocal(inv[:], s_ps[:])

        orow = op.tile([1, u], f32, tag="orow")
        nc.scalar.activation(orow[:], wsum[0:1, :], Act.Copy, scale=inv[:])
        nc.sync.dma_start(y[s:s + 1, :], orow[:])


def build_nc(b_loc=B_LOC, t_len=T, u=U, xbufs=10):
    key = (b_loc, t_len, u, xbufs)
    if key in _BUILD_CACHE:
        return _BUILD_CACHE[key]
    import concourse.bacc as bacc
    import concourse.tile as tile
    from concourse import mybir

    nc = bacc.Bacc(
        "TRN2",
        target_bir_lowering=False,
        debug=False,
        num_devices=N_CORES,
    )
    x = nc.dram_tensor("x", [b_loc, t_len, u], mybir.dt.float32, kind="ExternalInput").ap()
    W = nc.dram_tensor("W", [u, 1], mybir.dt.float32, kind="ExternalInput").ap()
    b = nc.dram_tensor("b", [t_len, 1], mybir.dt.float32, kind="ExternalInput").ap()
    y = nc.dram_tensor("y", [b_loc, u], mybir.dt.float32, kind="ExternalOutput").ap()

    with tile.TileContext(nc) as tc:
        with ExitStack() as ctx:
            _emit(ctx, tc, x, W, b, y, xbufs)
    nc.compile()
    _BUILD_CACHE[key] = nc
    return nc


def kernel(x, W, b):
    x = np.ascontiguousarray(np.asarray(x, dtype=np.float32))
    W = np.ascontiguousarray(np.asarray(W, dtype=np.float32))
    b = np.ascontiguousarray(np.asarray(b, dtype=np.float32))
    assert x.shape == (B, T, U), x.shape

    from concourse.bass_utils import run_bass_kernel_spmd

    nc = build_nc()
    in_maps = [
        {
            "x": np.ascontiguousarray(x[i * B_LOC:(i + 1) * B_LOC]),
            "W": W,
            "b": b,
        }
        for i in range(N_CORES)
    ]
    res = run_bass_kernel_spmd(nc, in_maps, core_ids=list(range(N_CORES)))
    return np.concatenate([r["y"] for r in res.results], axis=0)
